# revision 20
# baseline (speedup 1.0000x reference)
"""BiLSTM parser kernel for 8 TRN2 NeuronCores.

Strategy (per sharding hint): the sequential 2-layer BiLSTM is replicated on
every core (fwd/bwd interleaved on one core's engines); the n x n pair grid
is sharded by head rows: core i computes score[64*i : 64*i+64, :] using its
partition_id to slice the u-matrix columns. Host gathers the 8 row blocks.

Key structure:
  - feature-on-partition layouts throughout ([128 feat, 512 t] tiles).
  - input gates are matmul'd directly INTO PSUM in 64-step chunks
    (col = 8*(t%64)+j), so each step's recurrent matmuls accumulate on top
    and the sigmoid reads PSUM directly - no per-step adds or bias fixups.
  - recurrent matmuls: weights stationary in fp16 (FWL), h kept in an fp16
    shadow; gates land as 8 PSUM columns (i0,i1,f0,f1,o0,o1,g0,g1).
  - sigma trick: g-gate rows pre-scaled x2 on host; tanh(x)=2*sigmoid(2x)-1
    so ONE Sigmoid ACT covers all 8 gate columns.
  - biases ride a ones-row in the contraction (layer 0) or a rank-1 matmul
    (layer 1).
"""

import numpy as np

SEQ = 512
WDIM, PDIM = 256, 64
H = 256           # per-direction hidden
G = 4 * H         # 1024 gates
MLP = 512
NCORES = 8
HS = SEQ // NCORES  # 64 head rows per core
CH = 64             # LSTM steps per psum chunk

# psum col j <- weight col-slice start (i0,i1,f0,f1,o0,o1,g0,g1)
JCOL = [0, 128, 256, 384, 768, 896, 512, 640]

_prog_cache = {}


def _build_program(dtw):
    """Build the Bass program. dtw: mybir dtype for recurrent weights/h."""
    from contextlib import ExitStack

    import concourse.bass as bass
    import concourse.mybir as mybir
    import concourse.tile as tile
    from concourse import bacc

    dt = mybir.dt.float32
    T = SEQ
    nc = bacc.Bacc(
        "TRN2", target_bir_lowering=False, debug=False, num_devices=NCORES
    )

    # ---- DRAM I/O ----
    x0t = nc.dram_tensor("x0t", [384, T], dt, kind="ExternalInput")
    wih0t = nc.dram_tensor("wih0t", [2, 384, G], dt, kind="ExternalInput")
    whh0t = nc.dram_tensor("whh0t", [2, 256, G], dt, kind="ExternalInput")
    wih1t = nc.dram_tensor("wih1t", [2, 512, G], dt, kind="ExternalInput")
    whh1t = nc.dram_tensor("whh1t", [2, 256, G], dt, kind="ExternalInput")
    bc1r = nc.dram_tensor("bc1r", [2, G], dt, kind="ExternalInput")
    w1t = nc.dram_tensor("w1t", [512, MLP], dt, kind="ExternalInput")
    w2t = nc.dram_tensor("w2t", [512, MLP], dt, kind="ExternalInput")
    blin = nc.dram_tensor("blin", [128, 4], dt, kind="ExternalInput")
    wout = nc.dram_tensor("wout", [128, 4], dt, kind="ExternalInput")
    bout = nc.dram_tensor("bout", [128, 1], dt, kind="ExternalInput")
    outd = nc.dram_tensor("out", [HS, T], dt, kind="ExternalOutput")

    Sig = mybir.ActivationFunctionType.Sigmoid
    Tanh = mybir.ActivationFunctionType.Tanh

    with tile.TileContext(nc) as tc, ExitStack() as ctx:
        pers = ctx.enter_context(tc.tile_pool(name="pers", bufs=1))

        # layer outputs, one [128, 2T] tile per dir (k-halves side by side)
        l0m = [pers.tile([128, 2 * T], dt, tag=f"l0m{d}", name=f"l0m{d}")
               for d in range(2)]
        l1m = [pers.tile([128, 2 * T], dt, tag=f"l1m{d}", name=f"l1m{d}")
               for d in range(2)]
        l0 = [l0m[i // 2][:, T * (i % 2):T * (i % 2 + 1)] for i in range(4)]
        l1 = [l1m[i // 2][:, T * (i % 2):T * (i % 2 + 1)] for i in range(4)]

        ones_sb = pers.tile([1, T], dt, tag="ones_sb", name="ones_sb")
        nc.vector.memset(ones_sb[:], 1.0)

        def lstm_layer(xin, KT, wih_dram, whh_dram, biases, loutm, tag):
            """xin: list of KT [128, T] APs; biases: per-dir [1, G] AP/None.

            Per (dir, time-chunk tc): one PSUM bank [128, 8*CH] holds gate
            pre-activations for CH steps (col 8*(t%CH)+j), filled by the
            batched input matmuls; per-step recurrent matmuls accumulate.
            """
            with tc.tile_pool(name=f"w_{tag}", bufs=1) as wp, \
                 tc.tile_pool(name=f"s_{tag}", bufs=3) as sp, \
                 tc.tile_pool(name=f"c_{tag}", bufs=4) as cp, \
                 tc.tile_pool(name=f"ps_{tag}", bufs=2,
                              space=bass.MemorySpace.PSUM) as pp:
                wih_sb, whh_sb, hbf = [], [], []
                for d in range(2):
                    wi = wp.tile([128, KT * G], dt, tag=f"wih{d}",
                                 name=f"wih{d}")
                    for k in range(KT):
                        nc.sync.dma_start(
                            wi[:, G * k:G * (k + 1)],
                            wih_dram[d, 128 * k:128 * (k + 1), :])
                    wih_sb.append(wi)
                    w = wp.tile([128, 2 * G], dtw, tag=f"whh{d}",
                                name=f"whh{d}")
                    if dtw == dt:
                        for k in range(2):
                            nc.sync.dma_start(
                                w[:, G * k:G * (k + 1)],
                                whh_dram[d, 128 * k:128 * (k + 1), :])
                        hbf.append(loutm[d])
                    else:
                        wf = wp.tile([128, 2 * G], dt, tag="whh_stage",
                                     name="whh_stage")
                        for k in range(2):
                            nc.sync.dma_start(
                                wf[:, G * k:G * (k + 1)],
                                whh_dram[d, 128 * k:128 * (k + 1), :])
                        nc.vector.tensor_copy(w[:], wf[:])
                        hb = wp.tile([128, 2 * T], dtw, tag=f"hbf{d}",
                                     name=f"hbf{d}")
                        hbf.append(hb)
                    whh_sb.append(w)

                pg = {}   # (d, tc) -> psum tile

                def prefill(d, tc):
                    pt = pp.tile([128, 8 * CH], dt, name=f"pg{d}",
                                 tag=f"pg{d}")
                    pv = pt[:].rearrange("p (t j) -> p t j", j=8)
                    bias = biases[d] if biases is not None else None
                    for j in range(8):
                        for k in range(KT):
                            nc.tensor.matmul(
                                pv[:, :, j],
                                wih_sb[d][:, G * k + JCOL[j]:
                                          G * k + JCOL[j] + 128],
                                xin[k][:, CH * tc:CH * (tc + 1)],
                                start=(k == 0),
                                stop=(k == KT - 1 and bias is None),
                                skip_group_check=True)
                        if bias is not None:
                            nc.tensor.matmul(
                                pv[:, :, j],
                                bias[0:1, JCOL[j]:JCOL[j] + 128],
                                ones_sb[0:1, CH * tc:CH * (tc + 1)],
                                start=False, stop=True,
                                skip_group_check=True)
                    pg[(d, tc)] = pt

                def chunk_of(d, s):
                    tcol = s if d == 0 else SEQ - 1 - s
                    return tcol // CH

                for d in range(2):
                    prefill(d, chunk_of(d, 0))
                for d in range(2):
                    prefill(d, chunk_of(d, CH))

                cprev = [None, None]
                for t in range(SEQ):
                    if t % CH == 0 and t >= CH and t + CH < SEQ:
                        for d in range(2):
                            prefill(d, chunk_of(d, t + CH))
                    for d in range(2):
                        tcol = t if d == 0 else SEQ - 1 - t
                        tc, sl = tcol // CH, tcol % CH
                        pt = pg[(d, tc)]
                        if t > 0:
                            pcol = tcol - 1 if d == 0 else tcol + 1
                            hs = hbf[d]
                            for j in range(8):
                                for k in range(2):
                                    nc.tensor.matmul(
                                        pt[:, 8 * sl + j:8 * sl + j + 1],
                                        whh_sb[d][:, G * k + JCOL[j]:
                                                  G * k + JCOL[j] + 128],
                                        hs[:, T * k + pcol:T * k + pcol + 1],
                                        start=False, stop=(k == 1),
                                        skip_group_check=True)
                        sig = sp.tile([128, 8], dt, tag="sig", name="sig")
                        nc.scalar.activation(
                            sig[:], pt[:, 8 * sl:8 * sl + 8], Sig)
                        # tanh(x) = 2*sigmoid(2x)-1 (g rows pre-scaled x2)
                        nc.vector.tensor_scalar(
                            sig[:, 6:8], sig[:, 6:8], 2.0, -1.0,
                            mybir.AluOpType.mult, mybir.AluOpType.add)
                        t1 = cp.tile([128, 2], dt, tag="t1", name="t1")
                        nc.vector.tensor_mul(t1[:], sig[:, 0:2], sig[:, 6:8])
                        if t == 0:
                            cnew = t1
                        else:
                            cnew = cp.tile([128, 2], dt, tag="c", name="c")
                            nc.vector.tensor_mul(
                                cnew[:], sig[:, 2:4], cprev[d][:])
                            nc.vector.tensor_add(cnew[:], cnew[:], t1[:])
                        cprev[d] = cnew
                        tct = sp.tile([128, 2], dt, tag="tct", name="tct")
                        nc.scalar.activation(tct[:], cnew[:], Tanh)
                        hv = hbf[d][:].rearrange(
                            "p (k t) -> p t k", k=2)[:, tcol, :]
                        nc.vector.tensor_mul(hv, sig[:, 4:6], tct[:])
                if dtw != dt:
                    for d in range(2):
                        nc.vector.tensor_copy(loutm[d][:], hbf[d][:])

        # ---- layer 0 (bias rides the ones-row at x0t[320]) ----
        with tc.tile_pool(name="x0", bufs=1) as x0p:
            x0_sb = [x0p.tile([128, SEQ], dt, tag=f"x0_{k}", name=f"x0_{k}")
                     for k in range(3)]
            for k in range(3):
                nc.sync.dma_start(x0_sb[k][:], x0t[128 * k:128 * (k + 1), :])
            lstm_layer([s[:] for s in x0_sb], 3, wih0t, whh0t, None,
                       l0m, "l0")

        # ---- layer 1 (bias via rank-1 matmul) ----
        with tc.tile_pool(name="b1", bufs=1) as b1p:
            bc1_sb = b1p.tile([2, G], dt, tag="bc1_sb", name="bc1_sb")
            nc.sync.dma_start(bc1_sb[:], bc1r[:, :])
            lstm_layer(l0, 4, wih1t, whh1t,
                       [bc1_sb[0:1, :], bc1_sb[1:2, :]], l1m, "l1")

        # ---- u/v projections ----
        with tc.tile_pool(name="uv", bufs=1) as uvp, \
             tc.tile_pool(name="uvps", bufs=2,
                          space=bass.MemorySpace.PSUM) as uvpp:
            w1sb = uvp.tile([128, 4 * MLP], dt, tag="w1sb", name="w1sb")
            w2sb = uvp.tile([128, 4 * MLP], dt, tag="w2sb", name="w2sb")
            for k in range(4):
                nc.sync.dma_start(w1sb[:, MLP * k:MLP * (k + 1)],
                                  w1t[128 * k:128 * (k + 1), :])
                nc.sync.dma_start(w2sb[:, MLP * k:MLP * (k + 1)],
                                  w2t[128 * k:128 * (k + 1), :])
            blin_sb = uvp.tile([128, 4], dt, tag="blin_sb", name="blin_sb")
            nc.sync.dma_start(blin_sb[:], blin[:, :])
            u_sb = pers.tile([128, 4 * SEQ], dt, tag="u_sb", name="u_sb")
            v_sb = pers.tile([128, 4 * SEQ], dt, tag="v_sb", name="v_sb")
            for k in range(4):
                psu = uvpp.tile([128, SEQ], dt, tag="ups", name="ups")
                psv = uvpp.tile([128, SEQ], dt, tag="vps", name="vps")
                for dblk in range(4):
                    nc.tensor.matmul(
                        psu[:],
                        w1sb[:, MLP * dblk + 128 * k:
                             MLP * dblk + 128 * (k + 1)],
                        l1[dblk], start=(dblk == 0), stop=(dblk == 3))
                for dblk in range(4):
                    nc.tensor.matmul(
                        psv[:],
                        w2sb[:, MLP * dblk + 128 * k:
                             MLP * dblk + 128 * (k + 1)],
                        l1[dblk], start=(dblk == 0), stop=(dblk == 3))
                nc.vector.tensor_scalar_add(
                    u_sb[:, SEQ * k:SEQ * (k + 1)], psu[:],
                    blin_sb[:, k:k + 1])
                nc.vector.tensor_copy(
                    v_sb[:, SEQ * k:SEQ * (k + 1)], psv[:])

        # ---- per-core slice of u ----
        uloc = pers.tile([128, 4 * HS], dt, tag="uloc", name="uloc")
        pid = nc.sync.partition_id()
        for k in range(4):
            nc.sync.dma_start(
                uloc[:, HS * k:HS * (k + 1)],
                u_sb[:, bass.ds(SEQ * k + pid * HS, HS)])

        # ---- pair grid ----
        with tc.tile_pool(name="grid", bufs=1) as gp, \
             tc.tile_pool(name="pre", bufs=2) as prep, \
             tc.tile_pool(name="th", bufs=5) as thp, \
             tc.tile_pool(name="stg", bufs=2) as stp, \
             tc.tile_pool(name="gps", bufs=4,
                          space=bass.MemorySpace.PSUM) as gpp:
            wout_sb = gp.tile([128, 4], dt, tag="wout_sb", name="wout_sb")
            nc.sync.dma_start(wout_sb[:], wout[:, :])
            bout_sb = gp.tile([128, 1], dt, tag="bout_sb", name="bout_sb")
            nc.sync.dma_start(bout_sb[:], bout[:, :])
            for hg in range(HS // 4):
                ps = gpp.tile([128, SEQ], dt, name="grid_ps", tag="grid_ps")
                for j in range(4):
                    lh = 4 * hg + j
                    pre = prep.tile([128, 4 * SEQ], dt, name="pre", tag="pre")
                    for k in range(4):
                        nc.vector.tensor_scalar_add(
                            pre[:, SEQ * k:SEQ * (k + 1)],
                            v_sb[:, SEQ * k:SEQ * (k + 1)],
                            uloc[:, HS * k + lh:HS * k + lh + 1])
                    th = thp.tile([128, 4 * SEQ], dt, name="th", tag="th")
                    nc.scalar.activation(th[:], pre[:], Tanh)
                    for k in range(4):
                        nc.tensor.matmul(
                            ps[32 * j:32 * j + 1, :], wout_sb[:, k:k + 1],
                            th[:, SEQ * k:SEQ * (k + 1)],
                            start=(k == 0), stop=(k == 3),
                            skip_group_check=True,
                            tile_position=(0, 32 * j))
                stage = stp.tile([128, SEQ], dt, name="stage", tag="stage")
                nc.vector.tensor_scalar_add(stage[:], ps[:], bout_sb[:, 0:1])
                for j in range(4):
                    nc.sync.dma_start(
                        outd[4 * hg + j:4 * hg + j + 1, :],
                        stage[32 * j:32 * j + 1, :])

    nc.compile()
    return nc


def _prep_inputs(inputs):
    f = np.float32
    word_tensor = np.asarray(inputs["word_tensor"])
    pos_tensor = np.asarray(inputs["pos_tensor"])
    word_emb = np.asarray(inputs["word_emb"], f)
    pos_emb = np.asarray(inputs["pos_emb"], f)
    embeds = np.concatenate(
        [word_emb[word_tensor], pos_emb[pos_tensor]], axis=-1)  # [T, 320]

    x0t = np.zeros((384, SEQ), f)
    x0t[:320] = embeds.T
    x0t[320, :] = 1.0  # bias row

    wih0 = np.asarray(inputs["wih0"], f)
    wih0t = np.zeros((2, 384, G), f)
    b0 = np.asarray(inputs["bih0"], f) + np.asarray(inputs["bhh0"], f)
    for d in range(2):
        wih0t[d, :320] = wih0[d].T
        wih0t[d, 320, :] = b0[d]
    whh0t = np.ascontiguousarray(
        np.transpose(np.asarray(inputs["whh0"], f), (0, 2, 1)))
    wih1t = np.ascontiguousarray(
        np.transpose(np.asarray(inputs["wih1"], f), (0, 2, 1)))
    whh1t = np.ascontiguousarray(
        np.transpose(np.asarray(inputs["whh1"], f), (0, 2, 1)))
    bc1r = np.ascontiguousarray(
        np.asarray(inputs["bih1"], f) + np.asarray(inputs["bhh1"], f))
    # sigma trick: tanh(x) = 2*sigmoid(2x) - 1 -> g-gate weights/biases x2
    for wt in (wih0t, whh0t, wih1t, whh1t):
        wt[:, :, 512:768] *= 2.0
    bc1r[:, 512:768] *= 2.0

    W_lin = np.asarray(inputs["W_lin"], f)  # [MLP, 1024]
    w1t = np.ascontiguousarray(W_lin[:, :512].T)  # [512, MLP]
    w2t = np.ascontiguousarray(W_lin[:, 512:].T)
    b_lin = np.asarray(inputs["b_lin"], f)
    blin = np.zeros((128, 4), f)
    w_out = np.asarray(inputs["w_out"], f)
    wout = np.zeros((128, 4), f)
    for k in range(4):
        blin[:, k] = b_lin[128 * k:128 * (k + 1)]
        wout[:, k] = w_out[0, 128 * k:128 * (k + 1)]
    bout = np.broadcast_to(
        np.asarray(inputs["b_out"], f).reshape(1, 1), (128, 1)).copy()

    return {
        "x0t": x0t, "wih0t": wih0t, "whh0t": whh0t, "wih1t": wih1t,
        "whh1t": whh1t, "bc1r": bc1r, "w1t": w1t, "w2t": w2t,
        "blin": blin, "wout": wout, "bout": bout,
    }


def kernel(trace=False, **inputs):
    import os

    import concourse.mybir as mybir
    from concourse.bass_utils import run_bass_kernel_spmd

    key = os.environ.get("KERNEL_RECUR_DTYPE", "fp16")
    if key not in _prog_cache:
        dtw = {"f32": mybir.dt.float32, "bf16": mybir.dt.bfloat16,
               "fp16": mybir.dt.float16}[key]
        _prog_cache[key] = _build_program(dtw)
    nc = _prog_cache[key]

    in_map = _prep_inputs(inputs)
    res = run_bass_kernel_spmd(
        nc, [dict(in_map) for _ in range(NCORES)],
        core_ids=list(range(NCORES)), trace=trace)

    S = np.concatenate(
        [res.results[i]["out"] for i in range(NCORES)], axis=0)
    S = S.astype(np.float32)
    S[np.eye(SEQ, dtype=bool)] = 0.0
    if trace:
        return S, res
    return S
